# revision 1
# baseline (speedup 1.0000x reference)
"""Raw-bass embedding lookup for TRN2: out[i] = feature_array[int(x[i,0])].

Data-parallel over N across 8 NeuronCores; the [512, 64] table is replicated.
Host side converts the float case-IDs to int32 and pads each 25000-row shard
to 25088 = 128*196, laid out so SBUF partition p owns output rows
p*196 .. p*196+195.

A HW probe showed indirect InstDMACopy honors only one index per partition
(128 descriptors) per instruction, so each s-column is its own gather
(table rows land per-partition-contiguous in SBUF). Gathers pipeline through
a rotating 7-semaphore window (7 in flight stays under the 1024-descriptor
SWDGE ring); writebacks batch 28 s-columns into one contiguous-per-partition
HWDGE DMA (7KB/partition) once their gathers complete, overlapping later
gathers. Every instruction carries at most one semaphore wait (this walrus
build rejects more).
"""

import numpy as np

N = 200_000
C = 512
D = 64
NCORES = 8
NS = N // NCORES
P = 128
S = 196
SP = P * S
NSEM = 7
WB = 28  # s-columns per writeback (196 = 7*28); NSEM divides WB
NWB = S // WB

_RUN_OPTS: dict = {}
_LAST_RESULT = None
_LAST_IN_MAPS = None
_NC_CACHE = None


def _build():
    global _NC_CACHE
    if _NC_CACHE is not None:
        return _NC_CACHE
    import concourse.bass as bass
    import concourse.mybir as mybir
    from contextlib import ExitStack

    nc = bass.Bass()
    x = nc.dram_tensor("x", [P, S], mybir.dt.int32, kind="ExternalInput")
    feat = nc.dram_tensor("feature", [C, D], mybir.dt.float32, kind="ExternalInput")
    out = nc.dram_tensor("out", [SP, D], mybir.dt.float32, kind="ExternalOutput")
    out_v = out[:].rearrange("(p s) d -> p (s d)", p=P)

    with (
        ExitStack() as stack,
        nc.sbuf_tensor("xi", [P, S], mybir.dt.int32) as xi,
        nc.sbuf_tensor("g", [P, S * D], mybir.dt.float32) as g,
        nc.semaphore("s_load") as s_load,
        nc.Block() as block,
    ):
        s_gath = [stack.enter_context(nc.semaphore(f"s_g{k}")) for k in range(NSEM)]
        s_out = [stack.enter_context(nc.semaphore(f"s_o{k}")) for k in range(NWB)]

        @block.sync
        def _(sync):
            sync.dma_start(out=xi[:], in_=x[:]).then_inc(s_load, 16)
            for w in range(NWB):
                # window w covers s < 28*(w+1); each of the 7 sems has had
                # exactly 4*(w+1) increments of 16 by then
                for k in range(NSEM):
                    sync.wait_ge(s_gath[k], 16 * (WB // NSEM) * (w + 1))
                sync.dma_start(
                    out=out_v[:, w * WB * D : (w + 1) * WB * D],
                    in_=g[:, w * WB * D : (w + 1) * WB * D],
                ).then_inc(s_out[w], 16)
            for w in range(NWB):
                sync.wait_ge(s_out[w], 16)

        @block.gpsimd
        def _(gpsimd):
            gpsimd.wait_ge(s_load, 16)
            for s in range(S):
                k, r = s % NSEM, s // NSEM
                if r > 0:
                    gpsimd.wait_ge(s_gath[k], 16 * r)
                gpsimd.indirect_dma_start(
                    out=g[:, s * D : (s + 1) * D],
                    out_offset=None,
                    in_=feat[:],
                    in_offset=bass.IndirectOffsetOnAxis(
                        ap=xi[:, s : s + 1], axis=0
                    ),
                ).then_inc(s_gath[k], 16)

    _NC_CACHE = nc
    return nc


def kernel(x, feature_array):
    global _LAST_RESULT, _LAST_IN_MAPS
    from concourse.bass_utils import run_bass_kernel_spmd

    nc = _build()
    xs = np.asarray(x).reshape(NCORES, NS).astype(np.int32)
    feat = np.ascontiguousarray(np.asarray(feature_array, dtype=np.float32))
    in_maps = []
    for i in range(NCORES):
        xp = np.zeros((P, S), dtype=np.int32)
        xp.reshape(-1)[:NS] = xs[i]
        in_maps.append({"x": xp, "feature": feat})
    _LAST_IN_MAPS = in_maps
    res = run_bass_kernel_spmd(nc, in_maps, core_ids=list(range(NCORES)), **_RUN_OPTS)
    _LAST_RESULT = res
    return np.concatenate([r["out"][:NS] for r in res.results], axis=0)



# revision 7
# speedup vs baseline: 3.4951x; 3.4951x over previous
"""One-hot-matmul embedding lookup for TRN2: out[i] = feature_array[int(x[i,0])].

Data-parallel over N across 8 NeuronCores; the [512, 64] table is replicated.

Per core (25000 rows padded to 25088 = 196 tiles of 128 rows, in 25 groups
of 8 tiles = one PSUM bank each):
  - x (fp16, exact for ids < 2048) is DMA-broadcast to all 128 partitions
    (source partition-stride 0), one chunk per group, alternating between
    the gpsimd and sync DMA queues.
  - DVE builds the transposed one-hot oh[c, j*128+m] = (x[128(8g+j)+m] ==
    c + 128h) for the 4 case-chunks h via tensor_scalar(is_equal) against
    a per-partition fp32 iota column; fp16 SBUF operands hit the 4x DVE
    mode (~267ns per 1024-wide chunk compare).
  - PE accumulates psum[m, d] += oh_h^T @ feat_chunk_h (fp16, 64-col
    moving operand; 4 matmuls per 128-row tile).
  - The scalar engine drains each PSUM bank (8 tiles) to SBUF; writeback
    DMAs alternate between the gpsimd and sync queues.

Raw bass (not TileContext): this walrus build rejects instructions with
more than one semaphore wait, so cross-engine dependencies are split into
single-wait `wait_ge` sequencer NOPs on the consuming engine.

This replaces the previous gpsimd indirect-DMA gather (SWDGE descriptor
generation was 94% busy at ~9ns/row -> 294us). fp16 table rounding gives
rel err ~2^-11, far under the 2e-2 gate.
"""

import numpy as np

N = 200_000
C = 512
D = 64
NCORES = 8
NS = N // NCORES  # 25000
P = 128
T = 196  # tiles of 128 rows per core
NSP = P * T  # 25088 padded rows per core
G = 8  # tiles per PSUM bank / drain group
NG = (T + G - 1) // G  # 25 groups; last has 4 tiles
B_OH = 3  # one-hot buffers
B_PS = 4  # psum banks
B_OSB = 3  # output staging buffers

_RUN_OPTS: dict = {}
_LAST_RESULT = None
_LAST_IN_MAPS = None
_NC_CACHE = None


def _build():
    global _NC_CACHE
    if _NC_CACHE is not None:
        return _NC_CACHE
    import concourse.bass as bass
    import concourse.mybir as mybir
    from contextlib import ExitStack

    f16 = mybir.dt.float16
    f32 = mybir.dt.float32
    EQ = mybir.AluOpType.is_equal

    nc = bass.Bass()
    xb = nc.dram_tensor("xb", [1, NSP], f16, kind="ExternalInput")
    feat = nc.dram_tensor("feat", [P, 4 * D], f16, kind="ExternalInput")
    iotas = nc.dram_tensor("iotas", [P, 4], f32, kind="ExternalInput")
    out = nc.dram_tensor("out", [NSP, D], f32, kind="ExternalOutput")

    def sg_of(g):
        return min(G, T - g * G)

    with ExitStack() as ctx:
        sb = ctx.enter_context
        feat_sb = sb(nc.sbuf_tensor("feat_sb", [P, 4 * D], f16))
        iota_sb = sb(nc.sbuf_tensor("iota_sb", [P, 4], f32))
        xrep = sb(nc.sbuf_tensor("xrep", [P, NSP], f16))
        oh = [sb(nc.sbuf_tensor(f"oh{b}", [P, 4, G, P], f16)) for b in range(B_OH)]
        osb = [sb(nc.sbuf_tensor(f"osb{b}", [P, G, D], f32)) for b in range(B_OSB)]
        ps = [sb(nc.psum_tensor(f"ps{b}", [P, G * D], f32)) for b in range(B_PS)]

        s_in_g = sb(nc.semaphore("s_in_g"))
        s_in_s = sb(nc.semaphore("s_in_s"))
        s_cmp = sb(nc.semaphore("s_cmp"))
        s_mm = sb(nc.semaphore("s_mm"))
        s_drain = sb(nc.semaphore("s_drain"))
        s_wb_g = sb(nc.semaphore("s_wb_g"))
        s_wb_s = sb(nc.semaphore("s_wb_s"))

        block = ctx.enter_context(nc.Block())

        def out_view(g):
            x0, sg = g * G * P, sg_of(g)
            return out[x0 : x0 + sg * P, :].rearrange("(j p) d -> p j d", p=P)

        @block.sync
        def _(sync):
            sync.dma_start(out=feat_sb[:], in_=feat[:]).then_inc(s_in_s, 16)
            sync.dma_start(out=iota_sb[:], in_=iotas[:]).then_inc(s_in_s, 16)
            for g in range(1, NG, 2):
                c0, cn = g * G * P, sg_of(g) * P
                sync.dma_start(
                    out=xrep[:, c0 : c0 + cn],
                    in_=xb[0:1, c0 : c0 + cn].to_broadcast((P, cn)),
                ).then_inc(s_in_s, 16)
            for g in range(1, NG, 2):
                sync.wait_ge(s_drain, g + 1)
                sync.dma_start(
                    out=out_view(g), in_=osb[g % B_OSB][:, : sg_of(g), :]
                ).then_inc(s_wb_s, 16)

        @block.gpsimd
        def _(gpsimd):
            for g in range(0, NG, 2):
                c0, cn = g * G * P, sg_of(g) * P
                gpsimd.dma_start(
                    out=xrep[:, c0 : c0 + cn],
                    in_=xb[0:1, c0 : c0 + cn].to_broadcast((P, cn)),
                ).then_inc(s_in_g, 16)
            for g in range(0, NG, 2):
                gpsimd.wait_ge(s_drain, g + 1)
                gpsimd.dma_start(
                    out=out_view(g), in_=osb[g % B_OSB][:, : sg_of(g), :]
                ).then_inc(s_wb_g, 16)

        @block.vector
        def _(vector):
            vector.wait_ge(s_in_s, 32)  # feat + iota loaded
            for g in range(NG):
                sg = sg_of(g)
                if g % 2 == 0:
                    vector.wait_ge(s_in_g, 16 * (g // 2 + 1))
                else:
                    vector.wait_ge(s_in_s, 16 * (2 + (g + 1) // 2))
                if g >= B_OH:
                    vector.wait_ge(s_mm, g - B_OH + 1)
                x0 = g * G * P
                for h in range(4):
                    i = vector.tensor_scalar(
                        oh[g % B_OH][:, h, :sg, :],
                        xrep[:, x0 : x0 + sg * P],
                        iota_sb[:, h : h + 1],
                        None,
                        EQ,
                    )
                    if h == 3:
                        i.then_inc(s_cmp, 1)

        @block.tensor
        def _(tensor):
            tensor.wait_ge(s_in_s, 16)  # feat loaded
            for g in range(NG):
                sg = sg_of(g)
                if g >= B_PS:
                    tensor.wait_ge(s_drain, g - B_PS + 1)
                tensor.wait_ge(s_cmp, g + 1)
                for j in range(sg):
                    for h in range(4):
                        i = tensor.matmul(
                            ps[g % B_PS][:, j * D : (j + 1) * D],
                            oh[g % B_OH][:, h, j, :],
                            feat_sb[:, h * D : (h + 1) * D],
                            start=(h == 0),
                            stop=(h == 3),
                        )
                        if j == sg - 1 and h == 3:
                            i.then_inc(s_mm, 1)

        @block.scalar
        def _(scalar):
            for g in range(NG):
                sg = sg_of(g)
                if g >= B_OSB:
                    q = g - B_OSB
                    scalar.wait_ge(
                        s_wb_g if q % 2 == 0 else s_wb_s, 16 * (q // 2 + 1)
                    )
                scalar.wait_ge(s_mm, g + 1)
                scalar.copy(
                    osb[g % B_OSB][:, :sg, :],
                    ps[g % B_PS][:].rearrange("p (j d) -> p j d", d=D)[:, :sg, :],
                ).then_inc(s_drain, 1)

    _NC_CACHE = nc
    return nc


def kernel(x, feature_array):
    global _LAST_RESULT, _LAST_IN_MAPS
    from concourse.bass_utils import run_bass_kernel_spmd

    nc = _build()
    xs = np.asarray(x).reshape(-1).astype(np.float16)  # ids < 512: exact in fp16
    feat = np.asarray(feature_array, dtype=np.float32)
    # feat16[c2, h*64+d] = feat[128h + c2, d]
    feat16 = (
        feat.reshape(4, P, D).transpose(1, 0, 2).reshape(P, 4 * D).astype(np.float16)
    )
    iotas = np.arange(P, dtype=np.float32)[:, None] + np.arange(
        4, dtype=np.float32
    ) * P

    in_maps = []
    for i in range(NCORES):
        xp = np.zeros((1, NSP), dtype=np.float16)
        xp[0, :NS] = xs[i * NS : (i + 1) * NS]
        in_maps.append({"xb": xp, "feat": feat16, "iotas": iotas.astype(np.float32)})
    _LAST_IN_MAPS = in_maps
    res = run_bass_kernel_spmd(nc, in_maps, core_ids=list(range(NCORES)), **_RUN_OPTS)
    _LAST_RESULT = res
    return np.concatenate([r["out"][:NS] for r in res.results], axis=0)


# revision 8
# speedup vs baseline: 3.5325x; 1.0107x over previous
"""One-hot-matmul embedding lookup for TRN2: out[i] = feature_array[int(x[i,0])].

Data-parallel over N across 8 NeuronCores; the [512, 64] table is replicated.

Per core (25000 rows padded to 25088 = 196 tiles of 128 rows, in 25 groups
of 8 tiles = one PSUM bank each):
  - x is pre-replicated host-side to [128, 25088] fp16 (ids < 2048 are
    exact in fp16), so the SBUF load is a plain contiguous HWDGE DMA.
    A partition-stride-0 broadcast DMA lowers to SWDGE whose gpsimd
    descriptor generation (~1.2us/group) was the previous bottleneck.
  - DVE builds the transposed one-hot oh[c, j*128+m] = (x[128(8g+j)+m] ==
    c + 128h) for the 4 case-chunks h via tensor_tensor(is_equal) against
    a replicated fp16 iota table (all operands fp16+SBUF for the fast
    DVE mode).
  - PE accumulates psum[m, d] += oh_h^T @ feat_chunk_h (fp16, 64-col
    moving operand; 4 matmuls per 128-row tile).
  - The scalar engine drains each PSUM bank (8 tiles) to SBUF; x-chunk
    loads and writeback DMAs alternate between the gpsimd and sync queues.

Raw bass (not TileContext): this walrus build rejects instructions with
more than one semaphore wait, so cross-engine dependencies are split into
single-wait `wait_ge` sequencer NOPs on the consuming engine.

fp16 table rounding gives rel err ~2^-11, far under the 2e-2 gate.
"""

import numpy as np

N = 200_000
C = 512
D = 64
NCORES = 8
NS = N // NCORES  # 25000
P = 128
T = 196  # tiles of 128 rows per core
NSP = P * T  # 25088 padded rows per core
G = 8  # tiles per PSUM bank / drain group
GP = G * P  # columns per group
NG = (T + G - 1) // G  # 25 groups; last has 4 tiles
B_OH = 3  # one-hot buffers
B_PS = 4  # psum banks
B_OSB = 3  # output staging buffers

_RUN_OPTS: dict = {}
_LAST_RESULT = None
_LAST_IN_MAPS = None
_NC_CACHE = None


def _build():
    global _NC_CACHE
    if _NC_CACHE is not None:
        return _NC_CACHE
    import concourse.bass as bass
    import concourse.mybir as mybir
    from contextlib import ExitStack

    f16 = mybir.dt.float16
    f32 = mybir.dt.float32
    EQ = mybir.AluOpType.is_equal

    nc = bass.Bass()
    xr = nc.dram_tensor("xr", [P, NSP], f16, kind="ExternalInput")
    feat = nc.dram_tensor("feat", [P, 4 * D], f16, kind="ExternalInput")
    iot = nc.dram_tensor("iot", [P, 4 * GP], f16, kind="ExternalInput")
    out = nc.dram_tensor("out", [NSP, D], f32, kind="ExternalOutput")

    def sg_of(g):
        return min(G, T - g * G)

    with ExitStack() as ctx:
        sb = ctx.enter_context
        feat_sb = sb(nc.sbuf_tensor("feat_sb", [P, 4 * D], f16))
        iot_sb = sb(nc.sbuf_tensor("iot_sb", [P, 4 * GP], f16))
        xrep = sb(nc.sbuf_tensor("xrep", [P, NSP], f16))
        oh = [sb(nc.sbuf_tensor(f"oh{b}", [P, 4, G, P], f16)) for b in range(B_OH)]
        osb = [sb(nc.sbuf_tensor(f"osb{b}", [P, G, D], f32)) for b in range(B_OSB)]
        ps = [sb(nc.psum_tensor(f"ps{b}", [P, G * D], f32)) for b in range(B_PS)]

        s_in_g = sb(nc.semaphore("s_in_g"))
        s_in_s = sb(nc.semaphore("s_in_s"))
        s_cmp = sb(nc.semaphore("s_cmp"))
        s_mm = sb(nc.semaphore("s_mm"))
        s_drain = sb(nc.semaphore("s_drain"))
        s_wb_g = sb(nc.semaphore("s_wb_g"))
        s_wb_s = sb(nc.semaphore("s_wb_s"))

        block = ctx.enter_context(nc.Block())

        def out_view(g):
            x0, sg = g * GP, sg_of(g)
            return out[x0 : x0 + sg * P, :].rearrange("(j p) d -> p j d", p=P)

        @block.sync
        def _(sync):
            sync.dma_start(out=feat_sb[:], in_=feat[:]).then_inc(s_in_s, 16)
            sync.dma_start(out=iot_sb[:], in_=iot[:]).then_inc(s_in_s, 16)
            for g in range(1, NG, 2):
                c0, cn = g * GP, sg_of(g) * P
                sync.dma_start(
                    out=xrep[:, c0 : c0 + cn], in_=xr[:, c0 : c0 + cn]
                ).then_inc(s_in_s, 16)
            for g in range(1, NG, 2):
                sync.wait_ge(s_drain, g + 1)
                sync.dma_start(
                    out=out_view(g), in_=osb[g % B_OSB][:, : sg_of(g), :]
                ).then_inc(s_wb_s, 16)

        @block.gpsimd
        def _(gpsimd):
            for g in range(0, NG, 2):
                c0, cn = g * GP, sg_of(g) * P
                gpsimd.dma_start(
                    out=xrep[:, c0 : c0 + cn], in_=xr[:, c0 : c0 + cn]
                ).then_inc(s_in_g, 16)
            for g in range(0, NG, 2):
                gpsimd.wait_ge(s_drain, g + 1)
                gpsimd.dma_start(
                    out=out_view(g), in_=osb[g % B_OSB][:, : sg_of(g), :]
                ).then_inc(s_wb_g, 16)

        @block.vector
        def _(vector):
            vector.wait_ge(s_in_s, 32)  # feat + iota loaded
            for g in range(NG):
                sg = sg_of(g)
                if g % 2 == 0:
                    vector.wait_ge(s_in_g, 16 * (g // 2 + 1))
                else:
                    vector.wait_ge(s_in_s, 16 * (2 + (g + 1) // 2))
                if g >= B_OH:
                    vector.wait_ge(s_mm, g - B_OH + 1)
                x0 = g * GP
                for h in range(4):
                    i = vector.tensor_tensor(
                        out=oh[g % B_OH][:, h, :sg, :],
                        in0=xrep[:, x0 : x0 + sg * P],
                        in1=iot_sb[:, h * GP : h * GP + sg * P],
                        op=EQ,
                    )
                    if h == 3:
                        i.then_inc(s_cmp, 1)

        @block.tensor
        def _(tensor):
            tensor.wait_ge(s_in_s, 16)  # feat loaded
            for g in range(NG):
                sg = sg_of(g)
                if g >= B_PS:
                    tensor.wait_ge(s_drain, g - B_PS + 1)
                tensor.wait_ge(s_cmp, g + 1)
                for j in range(sg):
                    for h in range(4):
                        i = tensor.matmul(
                            ps[g % B_PS][:, j * D : (j + 1) * D],
                            oh[g % B_OH][:, h, j, :],
                            feat_sb[:, h * D : (h + 1) * D],
                            start=(h == 0),
                            stop=(h == 3),
                        )
                        if j == sg - 1 and h == 3:
                            i.then_inc(s_mm, 1)

        @block.scalar
        def _(scalar):
            for g in range(NG):
                sg = sg_of(g)
                if g >= B_OSB:
                    q = g - B_OSB
                    scalar.wait_ge(
                        s_wb_g if q % 2 == 0 else s_wb_s, 16 * (q // 2 + 1)
                    )
                scalar.wait_ge(s_mm, g + 1)
                scalar.copy(
                    osb[g % B_OSB][:, :sg, :],
                    ps[g % B_PS][:].rearrange("p (j d) -> p j d", d=D)[:, :sg, :],
                ).then_inc(s_drain, 1)

    _NC_CACHE = nc
    return nc


def kernel(x, feature_array):
    global _LAST_RESULT, _LAST_IN_MAPS
    from concourse.bass_utils import run_bass_kernel_spmd

    nc = _build()
    xs = np.asarray(x).reshape(-1).astype(np.float16)  # ids < 512: exact in fp16
    feat = np.asarray(feature_array, dtype=np.float32)
    # feat16[c2, h*64+d] = feat[128h + c2, d]
    feat16 = (
        feat.reshape(4, P, D).transpose(1, 0, 2).reshape(P, 4 * D).astype(np.float16)
    )
    # iot[c, h*GP + k] = c + 128h (constant along k)
    iot = np.broadcast_to(
        (
            np.arange(P, dtype=np.float32)[:, None, None]
            + np.arange(4, dtype=np.float32)[None, :, None] * P
        ),
        (P, 4, GP),
    ).reshape(P, 4 * GP).astype(np.float16)

    in_maps = []
    for i in range(NCORES):
        xp = np.zeros((1, NSP), dtype=np.float16)
        xp[0, :NS] = xs[i * NS : (i + 1) * NS]
        xrep = np.ascontiguousarray(np.broadcast_to(xp, (P, NSP)))
        in_maps.append({"xr": xrep, "feat": feat16, "iot": iot})
    _LAST_IN_MAPS = in_maps
    res = run_bass_kernel_spmd(nc, in_maps, core_ids=list(range(NCORES)), **_RUN_OPTS)
    _LAST_RESULT = res
    return np.concatenate([r["out"][:NS] for r in res.results], axis=0)


# revision 9
# speedup vs baseline: 3.7272x; 1.0551x over previous
"""One-hot-matmul embedding lookup for TRN2: out[i] = feature_array[int(x[i,0])].

Data-parallel over N across 8 NeuronCores; the [512, 64] table is replicated.

Per core (25000 rows padded to 25088 = 196 tiles of 128 rows, in 25 groups
of 8 tiles = one PSUM bank each):
  - x is pre-replicated host-side to [128, 25088] fp16 (ids < 2048 are
    exact in fp16), so the SBUF load is a plain contiguous HWDGE DMA.
    A partition-stride-0 broadcast DMA lowers to SWDGE whose gpsimd
    descriptor generation (~1.2us/group) was the previous bottleneck.
  - DVE builds the transposed one-hot oh[c, j*128+m] = (x[128(8g+j)+m] ==
    c + 128h) for the 4 case-chunks h via tensor_tensor(is_equal) against
    a replicated fp16 iota table (all operands fp16+SBUF for the fast
    DVE mode).
  - PE accumulates psum[m, d] += oh_h^T @ feat_chunk_h (fp16, 64-col
    moving operand; 4 matmuls per 128-row tile).
  - The scalar engine drains each PSUM bank (8 tiles) to SBUF; x-chunk
    loads and writeback DMAs alternate between the gpsimd and sync queues.

Raw bass (not TileContext): this walrus build rejects instructions with
more than one semaphore wait, so cross-engine dependencies are split into
single-wait `wait_ge` sequencer NOPs on the consuming engine.

fp16 table rounding gives rel err ~2^-11, far under the 2e-2 gate.
"""

import numpy as np

N = 200_000
C = 512
D = 64
NCORES = 8
NS = N // NCORES  # 25000
P = 128
T = 196  # tiles of 128 rows per core
NSP = P * T  # 25088 padded rows per core
G = 8  # tiles per PSUM bank / drain group
GP = G * P  # columns per group
NG = (T + G - 1) // G  # 25 groups; last has 4 tiles
B_OH = 3  # one-hot buffers
B_PS = 4  # psum banks
B_OSB = 3  # output staging buffers

_RUN_OPTS: dict = {}
_LAST_RESULT = None
_LAST_IN_MAPS = None
_NC_CACHE = None


def _build():
    global _NC_CACHE
    if _NC_CACHE is not None:
        return _NC_CACHE
    import concourse.bass as bass
    import concourse.mybir as mybir
    from contextlib import ExitStack

    f16 = mybir.dt.float16
    f32 = mybir.dt.float32
    EQ = mybir.AluOpType.is_equal

    nc = bass.Bass()
    xr = nc.dram_tensor("xr", [P, NSP], f16, kind="ExternalInput")
    feat = nc.dram_tensor("feat", [P, 4 * D], f16, kind="ExternalInput")
    iot = nc.dram_tensor("iot", [P, 4], f32, kind="ExternalInput")
    out = nc.dram_tensor("out", [NSP, D], f32, kind="ExternalOutput")

    def sg_of(g):
        return min(G, T - g * G)

    with ExitStack() as ctx:
        sb = ctx.enter_context
        feat_sb = sb(nc.sbuf_tensor("feat_sb", [P, 4 * D], f16))
        iot_sb = sb(nc.sbuf_tensor("iot_sb", [P, 4], f32))
        xrep = sb(nc.sbuf_tensor("xrep", [P, NSP], f16))
        oh = [sb(nc.sbuf_tensor(f"oh{b}", [P, 4, G, P], f16)) for b in range(B_OH)]
        osb = [sb(nc.sbuf_tensor(f"osb{b}", [P, G, D], f32)) for b in range(B_OSB)]
        ps = [sb(nc.psum_tensor(f"ps{b}", [P, G * D], f32)) for b in range(B_PS)]

        s_in_g = sb(nc.semaphore("s_in_g"))
        s_in_s = sb(nc.semaphore("s_in_s"))
        s_cmp = sb(nc.semaphore("s_cmp"))
        s_mm = sb(nc.semaphore("s_mm"))
        s_drain = sb(nc.semaphore("s_drain"))
        s_wb_g = sb(nc.semaphore("s_wb_g"))
        s_wb_s = sb(nc.semaphore("s_wb_s"))

        block = ctx.enter_context(nc.Block())

        def out_view(g):
            x0, sg = g * GP, sg_of(g)
            return out[x0 : x0 + sg * P, :].rearrange("(p j) d -> p j d", p=P)

        @block.sync
        def _(sync):
            sync.dma_start(out=feat_sb[:], in_=feat[:]).then_inc(s_in_s, 16)
            sync.dma_start(out=iot_sb[:], in_=iot[:]).then_inc(s_in_s, 16)
            for g in range(1, NG, 2):
                c0, cn = g * GP, sg_of(g) * P
                sync.dma_start(
                    out=xrep[:, c0 : c0 + cn], in_=xr[:, c0 : c0 + cn]
                ).then_inc(s_in_s, 16)
            for g in range(1, NG, 2):
                sync.wait_ge(s_drain, g + 1)
                sync.dma_start(
                    out=out_view(g), in_=osb[g % B_OSB][:, : sg_of(g), :]
                ).then_inc(s_wb_s, 16)

        @block.gpsimd
        def _(gpsimd):
            for g in range(0, NG, 2):
                c0, cn = g * GP, sg_of(g) * P
                gpsimd.dma_start(
                    out=xrep[:, c0 : c0 + cn], in_=xr[:, c0 : c0 + cn]
                ).then_inc(s_in_g, 16)
            for g in range(0, NG, 2):
                gpsimd.wait_ge(s_drain, g + 1)
                gpsimd.dma_start(
                    out=out_view(g), in_=osb[g % B_OSB][:, : sg_of(g), :]
                ).then_inc(s_wb_g, 16)

        @block.vector
        def _(vector):
            vector.wait_ge(s_in_s, 32)  # feat + iota loaded
            for g in range(NG):
                sg = sg_of(g)
                if g % 2 == 0:
                    vector.wait_ge(s_in_g, 16 * (g // 2 + 1))
                else:
                    vector.wait_ge(s_in_s, 16 * (2 + (g + 1) // 2))
                if g >= B_OH:
                    vector.wait_ge(s_mm, g - B_OH + 1)
                x0 = g * GP
                for h in range(4):
                    i = vector.tensor_scalar(
                        oh[g % B_OH][:, h, :sg, :],
                        xrep[:, x0 : x0 + sg * P],
                        iot_sb[:, h : h + 1],
                        None,
                        EQ,
                    )
                    if h == 3:
                        i.then_inc(s_cmp, 1)

        @block.tensor
        def _(tensor):
            tensor.wait_ge(s_in_s, 16)  # feat loaded
            for g in range(NG):
                sg = sg_of(g)
                if g >= B_PS:
                    tensor.wait_ge(s_drain, g - B_PS + 1)
                tensor.wait_ge(s_cmp, g + 1)
                for j in range(sg):
                    for h in range(4):
                        i = tensor.matmul(
                            ps[g % B_PS][:, j * D : (j + 1) * D],
                            oh[g % B_OH][:, h, j, :],
                            feat_sb[:, h * D : (h + 1) * D],
                            start=(h == 0),
                            stop=(h == 3),
                        )
                        if j == sg - 1 and h == 3:
                            i.then_inc(s_mm, 1)

        @block.scalar
        def _(scalar):
            for g in range(NG):
                sg = sg_of(g)
                if g >= B_OSB:
                    q = g - B_OSB
                    scalar.wait_ge(
                        s_wb_g if q % 2 == 0 else s_wb_s, 16 * (q // 2 + 1)
                    )
                scalar.wait_ge(s_mm, g + 1)
                scalar.copy(
                    osb[g % B_OSB][:, :sg, :],
                    ps[g % B_PS][:].rearrange("p (j d) -> p j d", d=D)[:, :sg, :],
                ).then_inc(s_drain, 1)

    _NC_CACHE = nc
    return nc


def kernel(x, feature_array):
    global _LAST_RESULT, _LAST_IN_MAPS
    from concourse.bass_utils import run_bass_kernel_spmd

    nc = _build()
    xs = np.asarray(x).reshape(-1).astype(np.float16)  # ids < 512: exact in fp16
    feat = np.asarray(feature_array, dtype=np.float32)
    # feat16[c2, h*64+d] = feat[128h + c2, d]
    feat16 = (
        feat.reshape(4, P, D).transpose(1, 0, 2).reshape(P, 4 * D).astype(np.float16)
    )
    iot = (
        np.arange(P, dtype=np.float32)[:, None]
        + np.arange(4, dtype=np.float32)[None, :] * P
    ).astype(np.float32)

    in_maps = []
    for i in range(NCORES):
        xp = np.zeros(NSP, dtype=np.float16)
        xp[:NS] = xs[i * NS : (i + 1) * NS]
        # within each group, transpose (p, j) -> (j, p) so the device's
        # writeback lands rows back in original order contiguously
        nfull = (NG - 1) * GP
        head = xp[:nfull].reshape(NG - 1, P, G).transpose(0, 2, 1).reshape(-1)
        tail = xp[nfull:].reshape(P, T - (NG - 1) * G).T.reshape(-1)
        xcol = np.concatenate([head, tail])[None, :]
        xrep = np.ascontiguousarray(np.broadcast_to(xcol, (P, NSP)))
        in_maps.append({"xr": xrep, "feat": feat16, "iot": iot})
    _LAST_IN_MAPS = in_maps
    res = run_bass_kernel_spmd(nc, in_maps, core_ids=list(range(NCORES)), **_RUN_OPTS)
    _LAST_RESULT = res
    return np.concatenate([r["out"][:NS] for r in res.results], axis=0)
